# revision 6
# baseline (speedup 1.0000x reference)
"""MiniMax MoE gate (sigmoid + bias, top-8, normalized weights) on 8 TRN2 cores.

Full inputs in, full outputs out. Data-parallel over tokens: each core gets
1024 of the 8192 tokens; gate weight and bias are replicated.

Strategy:
  - The host pre-transposes x (and W) and splits them into bf16 hi/lo pairs
    (x == hi + lo to within 2^-18 relative). Shipping x^T directly removes all
    on-chip transposes, and bf16 operands run the PE at 1 cycle/row with fast
    weight loads (vs 4 cycles/row + slow loads for fp32).
  - logits are accumulated in fp32 PSUM as the 3-term compensated product
    x_hi@W_hi + x_hi@W_lo + x_lo@W_hi, which lands at fp32-level accuracy
    (validated: same top-8 flip count as a full fp32 kernel).
  - The x^T hi/lo chunk tiles are the matmul stationary operand ([128 d x
    128 t], fast-weight-loaded), W^T chunks the moving operand, so logits come
    out [128 tokens x 64 experts] - the layout the top-8 epilogue needs.
  - sigmoid = 1/(1 + e^-l) via ACT exp (~2 ULP) + DVE reciprocal.
  - top-8 via the DVE's MAX8/FIND_INDEX8 (descending values, ties by
    ascending index - exactly jax.lax.top_k semantics).
  - per-slot raw scores via one fused scalar_tensor_tensor per slot:
    ssel_k = sum_e (biased == msel_k) * score; weights = ssel * recip(sum).
  - Tokens are processed in NH column groups so epilogues overlap the next
    group's matmul/DMA phase.
"""

import numpy as np
import ml_dtypes

import concourse.bacc as bacc
import concourse.mybir as mybir
from concourse.bass_utils import run_bass_kernel_spmd
from concourse.tile import TileContext

T, D, E, K = 8192, 4096, 64, 8
NCORES = 8
P = 128
NH = 4  # token column-groups per core (epilogue/matmul overlap granularity)
F32 = mybir.dt.float32
BF16 = mybir.dt.bfloat16
BF16_NP = ml_dtypes.bfloat16


def build_nc(ts):
    """Per-core program for a shard of `ts` tokens."""
    dc = D // P                 # 32 contraction chunks
    nh = NH if ts >= NH * P else max(1, ts // P)
    th = ts // nh               # tokens per column group
    ntg = th // P               # 128-token tiles per group

    nc = bacc.Bacc("TRN2", target_bir_lowering=False)
    xh = nc.dram_tensor("xt_hi", [D, ts], BF16, kind="ExternalInput")
    xl = nc.dram_tensor("xt_lo", [D, ts], BF16, kind="ExternalInput")
    wh = nc.dram_tensor("wt_hi", [D, E], BF16, kind="ExternalInput")
    wl = nc.dram_tensor("wt_lo", [D, E], BF16, kind="ExternalInput")
    b = nc.dram_tensor("bias", [1, E], F32, kind="ExternalInput")
    oi = nc.dram_tensor("out_idx", [ts, K], mybir.dt.int32, kind="ExternalOutput")
    ow = nc.dram_tensor("out_w", [ts, K], F32, kind="ExternalOutput")

    with TileContext(nc) as tc:
        with (
            tc.tile_pool(name="const", bufs=1) as cpool,
            tc.tile_pool(name="xin", bufs=6) as xpool,
            tc.tile_pool(name="epi", bufs=2) as epool,
            tc.tile_pool(name="plogit", bufs=2, space="PSUM") as plogit,
        ):
            bias_row = cpool.tile([1, E], F32)
            nc.sync.dma_start(out=bias_row, in_=b[:, :])
            bias_bc = cpool.tile([P, E], F32)
            nc.gpsimd.partition_broadcast(bias_bc, bias_row)

            wth = cpool.tile([P, dc, E], BF16)
            nc.sync.dma_start(
                out=wth, in_=wh[:, :].rearrange("(c p) e -> p c e", p=P)
            )
            wtl = cpool.tile([P, dc, E], BF16)
            nc.sync.dma_start(
                out=wtl, in_=wl[:, :].rearrange("(c p) e -> p c e", p=P)
            )

            for h in range(nh):
                csl = slice(h * th, (h + 1) * th)
                lgs = [
                    plogit.tile([P, E], F32, tag=f"lg{i}", name=f"lg_h{h}_{i}")
                    for i in range(ntg)
                ]
                for c in range(dc):
                    rsl = slice(c * P, (c + 1) * P)
                    xhc = xpool.tile([P, th], BF16, tag="xh")
                    nc.sync.dma_start(out=xhc, in_=xh[rsl, csl])
                    xlc = xpool.tile([P, th], BF16, tag="xl")
                    nc.sync.dma_start(out=xlc, in_=xl[rsl, csl])
                    for i, lg in enumerate(lgs):
                        tsl = slice(i * P, (i + 1) * P)
                        nc.tensor.matmul(
                            lg, xhc[:, tsl], wth[:, c, :],
                            start=(c == 0), stop=False,
                        )
                        nc.tensor.matmul(
                            lg, xhc[:, tsl], wtl[:, c, :],
                            start=False, stop=False,
                        )
                        nc.tensor.matmul(
                            lg, xlc[:, tsl], wth[:, c, :],
                            start=False, stop=(c == dc - 1),
                        )

                for i, lg in enumerate(lgs):
                    bt = h * ntg + i
                    # sigmoid = 1 / (1 + e^-l)
                    ex = epool.tile([P, E], F32, tag="ex")
                    nc.scalar.activation(
                        out=ex, in_=lg,
                        func=mybir.ActivationFunctionType.Exp, scale=-1.0,
                    )
                    den = epool.tile([P, E], F32, tag="den")
                    nc.vector.tensor_scalar_add(den, ex, 1.0)
                    sc = epool.tile([P, E], F32, tag="sc")
                    nc.vector.reciprocal(out=sc, in_=den)
                    bi = epool.tile([P, E], F32, tag="bi")
                    nc.vector.tensor_tensor(
                        out=bi, in0=sc, in1=bias_bc, op=mybir.AluOpType.add
                    )
                    msel = epool.tile([P, K], F32, tag="msel")
                    nc.vector.max(out=msel, in_=bi)
                    idxu = epool.tile([P, K], mybir.dt.uint32, tag="idxu")
                    nc.vector.max_index(out=idxu, in_max=msel, in_values=bi)
                    ssel = epool.tile([P, K], F32, tag="ssel")
                    for k in range(K):
                        scr = epool.tile([P, E], F32, tag="scr")
                        nc.vector.scalar_tensor_tensor(
                            out=scr,
                            in0=bi,
                            scalar=msel[:, k:k + 1],
                            in1=sc,
                            op0=mybir.AluOpType.is_equal,
                            op1=mybir.AluOpType.mult,
                            accum_out=ssel[:, k:k + 1],
                        )
                    ssum = epool.tile([P, 1], F32, tag="ssum")
                    nc.vector.tensor_reduce(
                        out=ssum, in_=ssel,
                        axis=mybir.AxisListType.X, op=mybir.AluOpType.add,
                    )
                    rsum = epool.tile([P, 1], F32, tag="rsum")
                    nc.vector.reciprocal(out=rsum, in_=ssum)
                    wo = epool.tile([P, K], F32, tag="wo")
                    nc.vector.tensor_scalar_mul(wo, ssel, rsum[:])
                    nc.sync.dma_start(
                        out=oi[bt * P:(bt + 1) * P, :],
                        in_=idxu[:].bitcast(mybir.dt.int32),
                    )
                    nc.sync.dma_start(out=ow[bt * P:(bt + 1) * P, :], in_=wo)

    nc.compile()
    return nc


_NC_CACHE = {}


def _get_nc(ts):
    if ts not in _NC_CACHE:
        _NC_CACHE[ts] = build_nc(ts)
    return _NC_CACHE[ts]


def _split_bf16_t(a):
    """Return (hi, lo) bf16 arrays with a.T == hi + lo to ~2^-18 rel."""
    at = np.ascontiguousarray(a.T.astype(np.float32))
    hi = at.astype(BF16_NP)
    lo = (at - hi.astype(np.float32)).astype(BF16_NP)
    return hi, lo


def prepare_in_maps(x, gate_weight, bias):
    x = np.asarray(x, dtype=np.float32)
    gw = np.asarray(gate_weight, dtype=np.float32)
    bb = np.ascontiguousarray(np.asarray(bias, dtype=np.float32)).reshape(1, E)

    xth, xtl = _split_bf16_t(x)          # [D, T]
    wth, wtl = _split_bf16_t(gw)         # [D, E]

    ts = T // NCORES
    in_maps = []
    for cid in range(NCORES):
        csl = slice(cid * ts, (cid + 1) * ts)
        in_maps.append({
            "xt_hi": np.ascontiguousarray(xth[:, csl]),
            "xt_lo": np.ascontiguousarray(xtl[:, csl]),
            "wt_hi": wth,
            "wt_lo": wtl,
            "bias": bb,
        })
    return in_maps


def kernel(x, gate_weight, bias):
    ts = T // NCORES
    nc = _get_nc(ts)
    in_maps = prepare_in_maps(x, gate_weight, bias)
    res = run_bass_kernel_spmd(nc, in_maps, core_ids=list(range(NCORES)))
    idx = np.concatenate([r["out_idx"] for r in res.results], axis=0)
    wts = np.concatenate([r["out_w"] for r in res.results], axis=0)
    return idx, wts


# revision 7
# speedup vs baseline: 2.4173x; 2.4173x over previous
"""MiniMax MoE gate (sigmoid + bias, top-8, normalized weights) on 8 TRN2 cores.

Full inputs in, full outputs out. Data-parallel over tokens: each core gets
1024 of the 8192 tokens; gate weight and bias are replicated.

Strategy:
  - The host pre-transposes x and W and splits them into bf16 hi/lo pairs
    (x == hi + lo to within ~2^-18 relative). logits are accumulated in fp32
    PSUM as the 3-term compensated product x_hi@W_hi + x_hi@W_lo + x_lo@W_hi,
    which lands at fp32-level accuracy (validated: identical top-8 flip count
    to a full fp32 kernel). bf16 operands run the PE at 1 cycle/row with fast
    weight loads; shipping x^T removes all on-chip transposes.
  - The host additionally lays x^T out so that each per-core, per-token-group
    SBUF tile is CONTIGUOUS PER PARTITION in DRAM: one DMA descriptor per
    partition (kilobytes each) instead of one per 512-byte row. This is what
    keeps HBM at full rate and the Sync sequencer (which serializes
    descriptor generation) off the critical path.
  - Tokens are interleaved within each group (output partition q of tile i
    holds token q*ntg + i) so the [ts, 8] outputs are also per-partition
    contiguous and each group needs just two output DMAs.
  - x^T hi/lo chunk tiles are the matmul stationary operand (fast weight
    load), W^T chunks the moving operand; logits come out [128 tok x 64 exp],
    the layout the top-8 epilogue needs.
  - sigmoid = 1/(1 + e^-l) via ACT exp (~2 ULP) + DVE reciprocal; top-8 via
    DVE MAX8/FIND_INDEX8 (descending, ties by ascending index — exactly
    jax.lax.top_k); per-slot raw scores via one fused scalar_tensor_tensor
    per slot; weights = ssel * recip(sum).
"""

import numpy as np
import ml_dtypes

import concourse.bacc as bacc
import concourse.mybir as mybir
from concourse.bass_utils import run_bass_kernel_spmd
from concourse.tile import TileContext

T, D, E, K = 8192, 4096, 64, 8
NCORES = 8
P = 128
NH = 4        # token groups per core (epilogue/matmul overlap granularity)
NSUB = 4      # x sub-loads per group (DMA/matmul pipelining granularity)
F32 = mybir.dt.float32
BF16 = mybir.dt.bfloat16
BF16_NP = ml_dtypes.bfloat16
DC = D // P   # 32 contraction chunks


def build_nc(ts):
    """Per-core program for a shard of `ts` tokens."""
    nh = NH if ts >= NH * P else max(1, ts // P)
    th = ts // nh              # tokens per group
    ntg = th // P              # 128-token tiles per group
    nsub = min(NSUB, DC)
    csub = DC // nsub          # chunks per x sub-load

    nc = bacc.Bacc("TRN2", target_bir_lowering=False)
    # host-tiled layouts (see prepare_in_maps): row p of group-block h holds
    # all of partition p's data for that group, contiguous.
    xhd = nc.dram_tensor("xt_hi", [nh * P, DC * th], BF16, kind="ExternalInput")
    xld = nc.dram_tensor("xt_lo", [nh * P, DC * th], BF16, kind="ExternalInput")
    whd = nc.dram_tensor("wt_hi", [P, DC * E], BF16, kind="ExternalInput")
    wld = nc.dram_tensor("wt_lo", [P, DC * E], BF16, kind="ExternalInput")
    b = nc.dram_tensor("bias", [1, E], F32, kind="ExternalInput")
    oi = nc.dram_tensor("out_idx", [ts, K], mybir.dt.int32, kind="ExternalOutput")
    ow = nc.dram_tensor("out_w", [ts, K], F32, kind="ExternalOutput")

    with TileContext(nc) as tc:
        with (
            tc.tile_pool(name="const", bufs=1) as cpool,
            tc.tile_pool(name="xin", bufs=2) as xpool,
            tc.tile_pool(name="epi", bufs=2) as epool,
            tc.tile_pool(name="outb", bufs=2) as opool,
            tc.tile_pool(name="plogit", bufs=2, space="PSUM") as plogit,
        ):
            bias_row = cpool.tile([1, E], F32)
            nc.sync.dma_start(out=bias_row, in_=b[:, :])
            bias_bc = cpool.tile([P, E], F32)
            nc.gpsimd.partition_broadcast(bias_bc, bias_row)

            wth = cpool.tile([P, DC, E], BF16)
            nc.sync.dma_start(out=wth, in_=whd[:, :])
            wtl = cpool.tile([P, DC, E], BF16)
            nc.sync.dma_start(out=wtl, in_=wld[:, :])

            for h in range(nh):
                xht = xpool.tile([P, DC, th], BF16, tag="xh", name=f"xh{h}")
                xlt = xpool.tile([P, DC, th], BF16, tag="xl", name=f"xl{h}")
                for s in range(nsub):
                    sl = slice(s * csub * th, (s + 1) * csub * th)
                    nc.sync.dma_start(
                        out=xht[:].rearrange("p c t -> p (c t)")[:, sl],
                        in_=xhd[h * P:(h + 1) * P, sl],
                    )
                    nc.sync.dma_start(
                        out=xlt[:].rearrange("p c t -> p (c t)")[:, sl],
                        in_=xld[h * P:(h + 1) * P, sl],
                    )

                lgs = [
                    plogit.tile([P, E], F32, tag=f"lg{i}", name=f"lg_h{h}_{i}")
                    for i in range(ntg)
                ]
                for c in range(DC):
                    for i, lg in enumerate(lgs):
                        tsl = slice(i * P, (i + 1) * P)
                        nc.tensor.matmul(
                            lg, xht[:, c, tsl], wth[:, c, :],
                            start=(c == 0), stop=False,
                        )
                        nc.tensor.matmul(
                            lg, xht[:, c, tsl], wtl[:, c, :],
                            start=False, stop=False,
                        )
                        nc.tensor.matmul(
                            lg, xlt[:, c, tsl], wth[:, c, :],
                            start=False, stop=(c == DC - 1),
                        )

                oidx = opool.tile([P, ntg, K], mybir.dt.uint32, name=f"oidx{h}")
                owgt = opool.tile([P, ntg, K], F32, name=f"owgt{h}")
                for i, lg in enumerate(lgs):
                    # sigmoid = 1 / (1 + e^-l)
                    ex = epool.tile([P, E], F32, tag="ex")
                    nc.scalar.activation(
                        out=ex, in_=lg,
                        func=mybir.ActivationFunctionType.Exp, scale=-1.0,
                    )
                    den = epool.tile([P, E], F32, tag="den")
                    nc.vector.tensor_scalar_add(den, ex, 1.0)
                    sc = epool.tile([P, E], F32, tag="sc")
                    nc.vector.reciprocal(out=sc, in_=den)
                    bi = epool.tile([P, E], F32, tag="bi")
                    nc.vector.tensor_tensor(
                        out=bi, in0=sc, in1=bias_bc, op=mybir.AluOpType.add
                    )
                    msel = epool.tile([P, K], F32, tag="msel")
                    nc.vector.max(out=msel, in_=bi)
                    nc.vector.max_index(
                        out=oidx[:, i, :], in_max=msel, in_values=bi
                    )
                    ssel = epool.tile([P, K], F32, tag="ssel")
                    for k in range(K):
                        scr = epool.tile([P, E], F32, tag="scr")
                        nc.vector.scalar_tensor_tensor(
                            out=scr,
                            in0=bi,
                            scalar=msel[:, k:k + 1],
                            in1=sc,
                            op0=mybir.AluOpType.is_equal,
                            op1=mybir.AluOpType.mult,
                            accum_out=ssel[:, k:k + 1],
                        )
                    ssum = epool.tile([P, 1], F32, tag="ssum")
                    nc.vector.tensor_reduce(
                        out=ssum, in_=ssel,
                        axis=mybir.AxisListType.X, op=mybir.AluOpType.add,
                    )
                    rsum = epool.tile([P, 1], F32, tag="rsum")
                    nc.vector.reciprocal(out=rsum, in_=ssum)
                    nc.vector.tensor_scalar_mul(owgt[:, i, :], ssel, rsum[:])

                # token at output partition q of tile i is h*th + q*ntg + i,
                # so rows of oi/ow group-slices are per-partition contiguous
                nc.sync.dma_start(
                    out=oi[h * th:(h + 1) * th, :].rearrange(
                        "(q i) k -> q i k", i=ntg
                    ),
                    in_=oidx[:].bitcast(mybir.dt.int32),
                )
                nc.sync.dma_start(
                    out=ow[h * th:(h + 1) * th, :].rearrange(
                        "(q i) k -> q i k", i=ntg
                    ),
                    in_=owgt,
                )

    nc.compile()
    return nc


_NC_CACHE = {}


def _get_nc(ts):
    if ts not in _NC_CACHE:
        _NC_CACHE[ts] = build_nc(ts)
    return _NC_CACHE[ts]


def _tile_xt(xs, nh, th, ntg):
    """[ts, D] fp32 -> [nh*P, DC*th] fp32 in the device layout.

    Group h, partition row p holds x[h*th + q*ntg + i, c*P + p] at flat
    column c*th + i*P + q.
    """
    ts = xs.shape[0]
    # [h, q, i, c, p] <- token h*th + q*ntg + i, feature c*P + p
    a = xs.reshape(nh, P, ntg, DC, P)          # [h, q, i, c, p]
    a = a.transpose(0, 4, 3, 2, 1)             # [h, p, c, i, q]
    return np.ascontiguousarray(a).reshape(nh * P, DC * th)


def prepare_in_maps(x, gate_weight, bias):
    x = np.asarray(x, dtype=np.float32)
    gw = np.asarray(gate_weight, dtype=np.float32)
    bb = np.ascontiguousarray(np.asarray(bias, dtype=np.float32)).reshape(1, E)

    ts = T // NCORES
    nh = NH if ts >= NH * P else max(1, ts // P)
    th = ts // nh
    ntg = th // P

    # W^T in device layout [P, DC*E]: [p, c*E + e] = W[e, c*P + p]
    wt = np.ascontiguousarray(gw.T.reshape(DC, P, E).transpose(1, 0, 2)).reshape(
        P, DC * E
    )
    wh = wt.astype(BF16_NP)
    wl = (wt - wh.astype(np.float32)).astype(BF16_NP)

    in_maps = []
    for cid in range(NCORES):
        xs = x[cid * ts:(cid + 1) * ts]
        xt = _tile_xt(xs, nh, th, ntg)
        xh = xt.astype(BF16_NP)
        xl = (xt - xh.astype(np.float32)).astype(BF16_NP)
        in_maps.append({
            "xt_hi": xh,
            "xt_lo": xl,
            "wt_hi": wh,
            "wt_lo": wl,
            "bias": bb,
        })
    return in_maps


def kernel(x, gate_weight, bias):
    ts = T // NCORES
    nc = _get_nc(ts)
    in_maps = prepare_in_maps(x, gate_weight, bias)
    res = run_bass_kernel_spmd(nc, in_maps, core_ids=list(range(NCORES)))
    idx = np.concatenate([r["out_idx"] for r in res.results], axis=0)
    wts = np.concatenate([r["out_w"] for r in res.results], axis=0)
    return idx, wts


# revision 9
# speedup vs baseline: 2.7142x; 1.1228x over previous
"""MiniMax MoE gate (sigmoid + bias, top-8, normalized weights) on 8 TRN2 cores.

Full inputs in, full outputs out. Data-parallel over tokens: each core gets
1024 of the 8192 tokens; gate weight and bias are replicated.

Strategy:
  - The host pre-transposes x and W and splits them into bf16 hi/lo pairs
    (x == hi + lo to within ~2^-18 relative). logits are accumulated in fp32
    PSUM as the 3-term compensated product x_hi@W_hi + x_hi@W_lo + x_lo@W_hi,
    which lands at fp32-level accuracy (validated: identical top-8 flip count
    to a full fp32 kernel). bf16 operands run the PE at 1 cycle/row with fast
    weight loads; shipping x^T removes all on-chip transposes.
  - The host additionally lays x^T out so that each per-core, per-token-group
    SBUF tile is CONTIGUOUS PER PARTITION in DRAM: one DMA descriptor per
    partition (kilobytes each) instead of one per 512-byte row. This is what
    keeps HBM at full rate and the Sync sequencer (which serializes
    descriptor generation) off the critical path.
  - Tokens are interleaved within each group (output partition q of tile i
    holds token q*ntg + i) so the [ts, 8] outputs are also per-partition
    contiguous and each group needs just two output DMAs.
  - x^T hi/lo chunk tiles are the matmul stationary operand (fast weight
    load), W^T chunks the moving operand; logits come out [128 tok x 64 exp],
    the layout the top-8 epilogue needs.
  - sigmoid = 1/(1 + e^-l) via ACT exp (~2 ULP) + DVE reciprocal; top-8 via
    DVE MAX8/FIND_INDEX8 (descending, ties by ascending index — exactly
    jax.lax.top_k); per-slot raw scores via one fused scalar_tensor_tensor
    per slot; weights = ssel * recip(sum).
"""

import numpy as np
import ml_dtypes

import concourse.bacc as bacc
import concourse.mybir as mybir
from concourse.bass_utils import run_bass_kernel_spmd
from concourse.tile import TileContext

T, D, E, K = 8192, 4096, 64, 8
NCORES = 8
P = 128
NH = 4        # token groups per core (epilogue/matmul overlap granularity)
NSUB = 4      # x sub-loads per group (DMA/matmul pipelining granularity)
F32 = mybir.dt.float32
BF16 = mybir.dt.bfloat16
BF16_NP = ml_dtypes.bfloat16
DC = D // P   # 32 contraction chunks


def build_nc(ts):
    """Per-core program for a shard of `ts` tokens."""
    nh = NH if ts >= NH * P else max(1, ts // P)
    th = ts // nh              # tokens per group
    ntg = th // P              # 128-token tiles per group
    nsub = min(NSUB, DC)
    csub = DC // nsub          # chunks per x sub-load

    nc = bacc.Bacc("TRN2", target_bir_lowering=False)
    # host-tiled layouts (see prepare_in_maps): row p of group-block h holds
    # all of partition p's data for that group, contiguous.
    xhd = nc.dram_tensor("xt_hi", [nh * P, DC * th], BF16, kind="ExternalInput")
    xld = nc.dram_tensor("xt_lo", [nh * P, DC * th], BF16, kind="ExternalInput")
    whd = nc.dram_tensor("wt_hi", [P, DC * E], BF16, kind="ExternalInput")
    wld = nc.dram_tensor("wt_lo", [P, DC * E], BF16, kind="ExternalInput")
    b = nc.dram_tensor("bias", [1, E], F32, kind="ExternalInput")
    oi = nc.dram_tensor("out_idx", [ts, K], mybir.dt.int32, kind="ExternalOutput")
    ow = nc.dram_tensor("out_w", [ts, K], F32, kind="ExternalOutput")

    with TileContext(nc) as tc:
        with (
            tc.tile_pool(name="const", bufs=1) as cpool,
            tc.tile_pool(name="xin", bufs=2) as xpool,
            tc.tile_pool(name="epi", bufs=2) as epool,
            tc.tile_pool(name="outb", bufs=2) as opool,
            tc.tile_pool(name="plogit", bufs=2, space="PSUM") as plogit,
        ):
            bias_row = cpool.tile([1, E], F32)
            nc.gpsimd.dma_start(out=bias_row, in_=b[:, :])
            bias_bc = cpool.tile([P, E], F32)
            nc.gpsimd.partition_broadcast(bias_bc, bias_row)

            # w loads + x lo-loads ride the Activation HWDGE queue, x hi-loads
            # the SP queue: two hardware queues in parallel.
            wth = cpool.tile([P, DC, E], BF16)
            nc.scalar.dma_start(out=wth, in_=whd[:, :])
            wtl = cpool.tile([P, DC, E], BF16)
            nc.scalar.dma_start(out=wtl, in_=wld[:, :])

            for h in range(nh):
                xht = xpool.tile([P, DC, th], BF16, tag="xh", name=f"xh{h}")
                xlt = xpool.tile([P, DC, th], BF16, tag="xl", name=f"xl{h}")
                for s in range(nsub):
                    sl = slice(s * csub * th, (s + 1) * csub * th)
                    nc.sync.dma_start(
                        out=xht[:].rearrange("p c t -> p (c t)")[:, sl],
                        in_=xhd[h * P:(h + 1) * P, sl],
                    )
                    nc.scalar.dma_start(
                        out=xlt[:].rearrange("p c t -> p (c t)")[:, sl],
                        in_=xld[h * P:(h + 1) * P, sl],
                    )

                lgs = [
                    plogit.tile([P, E], F32, tag=f"lg{i}", name=f"lg_h{h}_{i}")
                    for i in range(ntg)
                ]
                for c in range(DC):
                    for i, lg in enumerate(lgs):
                        tsl = slice(i * P, (i + 1) * P)
                        nc.tensor.matmul(
                            lg, xht[:, c, tsl], wth[:, c, :],
                            start=(c == 0), stop=False,
                        )
                        nc.tensor.matmul(
                            lg, xht[:, c, tsl], wtl[:, c, :],
                            start=False, stop=False,
                        )
                        nc.tensor.matmul(
                            lg, xlt[:, c, tsl], wth[:, c, :],
                            start=False, stop=(c == DC - 1),
                        )

                oidx = opool.tile([P, ntg, K], mybir.dt.uint32, name=f"oidx{h}")
                owgt = opool.tile([P, ntg, K], F32, name=f"owgt{h}")
                for i, lg in enumerate(lgs):
                    # sigmoid = 1 / (1 + e^-l)
                    ex = epool.tile([P, E], F32, tag="ex")
                    nc.scalar.activation(
                        out=ex, in_=lg,
                        func=mybir.ActivationFunctionType.Exp, scale=-1.0,
                    )
                    den = epool.tile([P, E], F32, tag="den")
                    nc.vector.tensor_scalar_add(den, ex, 1.0)
                    sc = epool.tile([P, E], F32, tag="sc")
                    nc.vector.reciprocal(out=sc, in_=den)
                    bi = epool.tile([P, E], F32, tag="bi")
                    nc.vector.tensor_tensor(
                        out=bi, in0=sc, in1=bias_bc, op=mybir.AluOpType.add
                    )
                    msel = epool.tile([P, K], F32, tag="msel")
                    nc.vector.max(out=msel, in_=bi)
                    nc.vector.max_index(
                        out=oidx[:, i, :], in_max=msel, in_values=bi
                    )
                    ssel = epool.tile([P, K], F32, tag="ssel")
                    for k in range(K):
                        scr = epool.tile([P, E], F32, tag="scr")
                        nc.vector.scalar_tensor_tensor(
                            out=scr,
                            in0=bi,
                            scalar=msel[:, k:k + 1],
                            in1=sc,
                            op0=mybir.AluOpType.is_equal,
                            op1=mybir.AluOpType.mult,
                            accum_out=ssel[:, k:k + 1],
                        )
                    ssum = epool.tile([P, 1], F32, tag="ssum")
                    nc.vector.tensor_reduce(
                        out=ssum, in_=ssel,
                        axis=mybir.AxisListType.X, op=mybir.AluOpType.add,
                    )
                    rsum = epool.tile([P, 1], F32, tag="rsum")
                    nc.vector.reciprocal(out=rsum, in_=ssum)
                    nc.vector.tensor_scalar_mul(owgt[:, i, :], ssel, rsum[:])

                # token at output partition q of tile i is h*th + q*ntg + i,
                # so rows of oi/ow group-slices are per-partition contiguous
                nc.gpsimd.dma_start(
                    out=oi[h * th:(h + 1) * th, :].rearrange(
                        "(q i) k -> q i k", i=ntg
                    ),
                    in_=oidx[:].bitcast(mybir.dt.int32),
                )
                nc.gpsimd.dma_start(
                    out=ow[h * th:(h + 1) * th, :].rearrange(
                        "(q i) k -> q i k", i=ntg
                    ),
                    in_=owgt,
                )

    nc.compile()
    return nc


_NC_CACHE = {}


def _get_nc(ts):
    if ts not in _NC_CACHE:
        _NC_CACHE[ts] = build_nc(ts)
    return _NC_CACHE[ts]


def _tile_xt(xs, nh, th, ntg):
    """[ts, D] fp32 -> [nh*P, DC*th] fp32 in the device layout.

    Group h, partition row p holds x[h*th + q*ntg + i, c*P + p] at flat
    column c*th + i*P + q.
    """
    ts = xs.shape[0]
    # [h, q, i, c, p] <- token h*th + q*ntg + i, feature c*P + p
    a = xs.reshape(nh, P, ntg, DC, P)          # [h, q, i, c, p]
    a = a.transpose(0, 4, 3, 2, 1)             # [h, p, c, i, q]
    return np.ascontiguousarray(a).reshape(nh * P, DC * th)


def prepare_in_maps(x, gate_weight, bias):
    x = np.asarray(x, dtype=np.float32)
    gw = np.asarray(gate_weight, dtype=np.float32)
    bb = np.ascontiguousarray(np.asarray(bias, dtype=np.float32)).reshape(1, E)

    ts = T // NCORES
    nh = NH if ts >= NH * P else max(1, ts // P)
    th = ts // nh
    ntg = th // P

    # W^T in device layout [P, DC*E]: [p, c*E + e] = W[e, c*P + p]
    wt = np.ascontiguousarray(gw.T.reshape(DC, P, E).transpose(1, 0, 2)).reshape(
        P, DC * E
    )
    wh = wt.astype(BF16_NP)
    wl = (wt - wh.astype(np.float32)).astype(BF16_NP)

    in_maps = []
    for cid in range(NCORES):
        xs = x[cid * ts:(cid + 1) * ts]
        xt = _tile_xt(xs, nh, th, ntg)
        xh = xt.astype(BF16_NP)
        xl = (xt - xh.astype(np.float32)).astype(BF16_NP)
        in_maps.append({
            "xt_hi": xh,
            "xt_lo": xl,
            "wt_hi": wh,
            "wt_lo": wl,
            "bias": bb,
        })
    return in_maps


def kernel(x, gate_weight, bias):
    ts = T // NCORES
    nc = _get_nc(ts)
    in_maps = prepare_in_maps(x, gate_weight, bias)
    res = run_bass_kernel_spmd(nc, in_maps, core_ids=list(range(NCORES)))
    idx = np.concatenate([r["out_idx"] for r in res.results], axis=0)
    wts = np.concatenate([r["out_w"] for r in res.results], axis=0)
    return idx, wts


# revision 10
# speedup vs baseline: 2.7997x; 1.0315x over previous
"""MiniMax MoE gate (sigmoid + bias, top-8, normalized weights) on 8 TRN2 cores.

Full inputs in, full outputs out. Data-parallel over tokens: each core gets
1024 of the 8192 tokens; gate weight and bias are replicated.

Strategy:
  - The host pre-transposes x and W and splits them into bf16 hi/lo pairs
    (x == hi + lo to within ~2^-18 relative). logits are accumulated in fp32
    PSUM as the 3-term compensated product x_hi@W_hi + x_hi@W_lo + x_lo@W_hi,
    which lands at fp32-level accuracy (validated: identical top-8 flip count
    to a full fp32 kernel). bf16 operands run the PE at 1 cycle/row with fast
    weight loads; shipping x^T removes all on-chip transposes.
  - The host additionally lays x^T out so that each per-core, per-token-group
    SBUF tile is CONTIGUOUS PER PARTITION in DRAM: one DMA descriptor per
    partition (kilobytes each) instead of one per 512-byte row. This is what
    keeps HBM at full rate and the Sync sequencer (which serializes
    descriptor generation) off the critical path.
  - Tokens are interleaved within each group (output partition q of tile i
    holds token q*ntg + i) so the [ts, 8] outputs are also per-partition
    contiguous and each group needs just two output DMAs.
  - x^T hi/lo chunk tiles are the matmul stationary operand (fast weight
    load), W^T chunks the moving operand; logits come out [128 tok x 64 exp],
    the layout the top-8 epilogue needs.
  - sigmoid = 1/(1 + e^-l) via ACT exp (~2 ULP) + DVE reciprocal; top-8 via
    DVE MAX8/FIND_INDEX8 (descending, ties by ascending index — exactly
    jax.lax.top_k); per-slot raw scores via one fused scalar_tensor_tensor
    per slot; weights = ssel * recip(sum).
"""

import numpy as np
import ml_dtypes

import concourse.bacc as bacc
import concourse.mybir as mybir
from concourse.bass_utils import run_bass_kernel_spmd
from concourse.tile import TileContext

T, D, E, K = 8192, 4096, 64, 8
NCORES = 8
P = 128
import os
NH = int(os.environ.get('KNH', '4'))  # token groups per core
NSUB = int(os.environ.get('KNSUB', '4'))  # x sub-loads per group
F32 = mybir.dt.float32
BF16 = mybir.dt.bfloat16
BF16_NP = ml_dtypes.bfloat16
DC = D // P   # 32 contraction chunks


def build_nc(ts):
    """Per-core program for a shard of `ts` tokens."""
    nh = NH if ts >= NH * P else max(1, ts // P)
    th = ts // nh              # tokens per group
    ntg = th // P              # 128-token tiles per group
    nsub = min(NSUB, DC)
    csub = DC // nsub          # chunks per x sub-load

    nc = bacc.Bacc("TRN2", target_bir_lowering=False)
    # host-tiled layouts (see prepare_in_maps): row p of group-block h holds
    # all of partition p's data for that group, contiguous.
    xhd = nc.dram_tensor("xt_hi", [nh * P, DC * th], BF16, kind="ExternalInput")
    xld = nc.dram_tensor("xt_lo", [nh * P, DC * th], BF16, kind="ExternalInput")
    whd = nc.dram_tensor("wt_hi", [P, DC * E], BF16, kind="ExternalInput")
    wld = nc.dram_tensor("wt_lo", [P, DC * E], BF16, kind="ExternalInput")
    b = nc.dram_tensor("bias", [1, E], F32, kind="ExternalInput")
    oi = nc.dram_tensor("out_idx", [ts, K], mybir.dt.int32, kind="ExternalOutput")
    ow = nc.dram_tensor("out_w", [ts, K], F32, kind="ExternalOutput")

    with TileContext(nc) as tc:
        with (
            tc.tile_pool(name="const", bufs=1) as cpool,
            tc.tile_pool(name="xin", bufs=2) as xpool,
            tc.tile_pool(name="epi", bufs=2) as epool,
            tc.tile_pool(name="outb", bufs=2) as opool,
            tc.tile_pool(name="plogit", bufs=2, space="PSUM") as plogit,
        ):
            bias_row = cpool.tile([1, E], F32)
            nc.gpsimd.dma_start(out=bias_row, in_=b[:, :])
            bias_bc = cpool.tile([P, E], F32)
            nc.gpsimd.partition_broadcast(bias_bc, bias_row)

            # w loads + x lo-loads ride the Activation HWDGE queue, x hi-loads
            # the SP queue: two hardware queues in parallel.
            wth = cpool.tile([P, DC, E], BF16)
            nc.scalar.dma_start(out=wth, in_=whd[:, :])
            wtl = cpool.tile([P, DC, E], BF16)
            nc.scalar.dma_start(out=wtl, in_=wld[:, :])

            for h in range(nh):
                xht = xpool.tile([P, DC, th], BF16, tag="xh", name=f"xh{h}")
                xlt = xpool.tile([P, DC, th], BF16, tag="xl", name=f"xl{h}")
                for s in range(nsub):
                    sl = slice(s * csub * th, (s + 1) * csub * th)
                    nc.sync.dma_start(
                        out=xht[:].rearrange("p c t -> p (c t)")[:, sl],
                        in_=xhd[h * P:(h + 1) * P, sl],
                    )
                    nc.scalar.dma_start(
                        out=xlt[:].rearrange("p c t -> p (c t)")[:, sl],
                        in_=xld[h * P:(h + 1) * P, sl],
                    )

                lgs = [
                    plogit.tile([P, E], F32, tag=f"lg{i}", name=f"lg_h{h}_{i}")
                    for i in range(ntg)
                ]
                for c in range(DC):
                    for i, lg in enumerate(lgs):
                        tsl = slice(i * P, (i + 1) * P)
                        nc.tensor.matmul(
                            lg, xht[:, c, tsl], wth[:, c, :],
                            start=(c == 0), stop=False,
                        )
                        nc.tensor.matmul(
                            lg, xht[:, c, tsl], wtl[:, c, :],
                            start=False, stop=False,
                        )
                        nc.tensor.matmul(
                            lg, xlt[:, c, tsl], wth[:, c, :],
                            start=False, stop=(c == DC - 1),
                        )

                oidx = opool.tile([P, ntg, K], mybir.dt.uint32, name=f"oidx{h}")
                owgt = opool.tile([P, ntg, K], F32, name=f"owgt{h}")
                for i, lg in enumerate(lgs):
                    # sigmoid = 1 / (1 + e^-l)
                    ex = epool.tile([P, E], F32, tag="ex")
                    nc.scalar.activation(
                        out=ex, in_=lg,
                        func=mybir.ActivationFunctionType.Exp, scale=-1.0,
                    )
                    den = epool.tile([P, E], F32, tag="den")
                    nc.vector.tensor_scalar_add(den, ex, 1.0)
                    sc = epool.tile([P, E], F32, tag="sc")
                    nc.vector.reciprocal(out=sc, in_=den)
                    bi = epool.tile([P, E], F32, tag="bi")
                    nc.vector.tensor_tensor(
                        out=bi, in0=sc, in1=bias_bc, op=mybir.AluOpType.add
                    )
                    msel = epool.tile([P, K], F32, tag="msel")
                    nc.vector.max(out=msel, in_=bi)
                    nc.vector.max_index(
                        out=oidx[:, i, :], in_max=msel, in_values=bi
                    )
                    ssel = epool.tile([P, K], F32, tag="ssel")
                    for k in range(K):
                        scr = epool.tile([P, E], F32, tag="scr")
                        nc.vector.scalar_tensor_tensor(
                            out=scr,
                            in0=bi,
                            scalar=msel[:, k:k + 1],
                            in1=sc,
                            op0=mybir.AluOpType.is_equal,
                            op1=mybir.AluOpType.mult,
                            accum_out=ssel[:, k:k + 1],
                        )
                    ssum = epool.tile([P, 1], F32, tag="ssum")
                    nc.vector.tensor_reduce(
                        out=ssum, in_=ssel,
                        axis=mybir.AxisListType.X, op=mybir.AluOpType.add,
                    )
                    rsum = epool.tile([P, 1], F32, tag="rsum")
                    nc.vector.reciprocal(out=rsum, in_=ssum)
                    nc.vector.tensor_scalar_mul(owgt[:, i, :], ssel, rsum[:])

                # token at output partition q of tile i is h*th + q*ntg + i,
                # so rows of oi/ow group-slices are per-partition contiguous
                nc.gpsimd.dma_start(
                    out=oi[h * th:(h + 1) * th, :].rearrange(
                        "(q i) k -> q i k", i=ntg
                    ),
                    in_=oidx[:].bitcast(mybir.dt.int32),
                )
                nc.gpsimd.dma_start(
                    out=ow[h * th:(h + 1) * th, :].rearrange(
                        "(q i) k -> q i k", i=ntg
                    ),
                    in_=owgt,
                )

    nc.compile()
    return nc


_NC_CACHE = {}


def _get_nc(ts):
    if ts not in _NC_CACHE:
        _NC_CACHE[ts] = build_nc(ts)
    return _NC_CACHE[ts]


def _tile_xt(xs, nh, th, ntg):
    """[ts, D] fp32 -> [nh*P, DC*th] fp32 in the device layout.

    Group h, partition row p holds x[h*th + q*ntg + i, c*P + p] at flat
    column c*th + i*P + q.
    """
    ts = xs.shape[0]
    # [h, q, i, c, p] <- token h*th + q*ntg + i, feature c*P + p
    a = xs.reshape(nh, P, ntg, DC, P)          # [h, q, i, c, p]
    a = a.transpose(0, 4, 3, 2, 1)             # [h, p, c, i, q]
    return np.ascontiguousarray(a).reshape(nh * P, DC * th)


def prepare_in_maps(x, gate_weight, bias):
    x = np.asarray(x, dtype=np.float32)
    gw = np.asarray(gate_weight, dtype=np.float32)
    bb = np.ascontiguousarray(np.asarray(bias, dtype=np.float32)).reshape(1, E)

    ts = T // NCORES
    nh = NH if ts >= NH * P else max(1, ts // P)
    th = ts // nh
    ntg = th // P

    # W^T in device layout [P, DC*E]: [p, c*E + e] = W[e, c*P + p]
    wt = np.ascontiguousarray(gw.T.reshape(DC, P, E).transpose(1, 0, 2)).reshape(
        P, DC * E
    )
    wh = wt.astype(BF16_NP)
    wl = (wt - wh.astype(np.float32)).astype(BF16_NP)

    in_maps = []
    for cid in range(NCORES):
        xs = x[cid * ts:(cid + 1) * ts]
        xt = _tile_xt(xs, nh, th, ntg)
        xh = xt.astype(BF16_NP)
        xl = (xt - xh.astype(np.float32)).astype(BF16_NP)
        in_maps.append({
            "xt_hi": xh,
            "xt_lo": xl,
            "wt_hi": wh,
            "wt_lo": wl,
            "bias": bb,
        })
    return in_maps


def kernel(x, gate_weight, bias):
    ts = T // NCORES
    nc = _get_nc(ts)
    in_maps = prepare_in_maps(x, gate_weight, bias)
    res = run_bass_kernel_spmd(nc, in_maps, core_ids=list(range(NCORES)))
    idx = np.concatenate([r["out_idx"] for r in res.results], axis=0)
    wts = np.concatenate([r["out_w"] for r in res.results], axis=0)
    return idx, wts


# revision 13
# speedup vs baseline: 2.9440x; 1.0515x over previous
"""MiniMax MoE gate (sigmoid + bias, top-8, normalized weights) on 8 TRN2 cores.

Full inputs in, full outputs out. Data-parallel over tokens: each core gets
1024 of the 8192 tokens; gate weight and bias are replicated.

Strategy:
  - The host pre-transposes x and W and splits them into bf16 hi/lo pairs
    (x == hi + lo to within ~2^-18 relative). logits are accumulated in fp32
    PSUM as the 3-term compensated product x_hi@W_hi + x_hi@W_lo + x_lo@W_hi,
    which lands at fp32-level accuracy (validated: identical top-8 flip count
    to a full fp32 kernel). bf16 operands run the PE at 1 cycle/row with fast
    weight loads; shipping x^T removes all on-chip transposes.
  - The host additionally lays x^T out so that each per-core, per-token-group
    SBUF tile is CONTIGUOUS PER PARTITION in DRAM: one DMA descriptor per
    partition (kilobytes each) instead of one per 512-byte row. This is what
    keeps HBM at full rate and the Sync sequencer (which serializes
    descriptor generation) off the critical path.
  - Tokens are interleaved within each group (output partition q of tile i
    holds token q*ntg + i) so the [ts, 8] outputs are also per-partition
    contiguous and each group needs just two output DMAs.
  - x^T hi/lo chunk tiles are the matmul stationary operand (fast weight
    load), W^T chunks the moving operand; logits come out [128 tok x 64 exp],
    the layout the top-8 epilogue needs.
  - sigmoid = 1/(1 + e^-l) via ACT exp (~2 ULP) + DVE reciprocal; top-8 via
    DVE MAX8/FIND_INDEX8 (descending, ties by ascending index — exactly
    jax.lax.top_k); per-slot raw scores via one fused scalar_tensor_tensor
    per slot; weights = ssel * recip(sum).
"""

import numpy as np
import ml_dtypes

import concourse.bacc as bacc
import concourse.mybir as mybir
from concourse.bass_utils import run_bass_kernel_spmd
from concourse.tile import TileContext

T, D, E, K = 8192, 4096, 64, 8
NCORES = 8
P = 128
import os
NH = int(os.environ.get('KNH', '4'))  # token groups per core
NSUB = int(os.environ.get('KNSUB', '4'))  # x sub-loads per group
F32 = mybir.dt.float32
BF16 = mybir.dt.bfloat16
BF16_NP = ml_dtypes.bfloat16
DC = D // P   # 32 contraction chunks


def build_nc(ts):
    """Per-core program for a shard of `ts` tokens."""
    nh = NH if ts >= NH * P else max(1, ts // P)
    th = ts // nh              # tokens per group
    ntg = th // P              # 128-token tiles per group
    nsub = min(NSUB, DC)
    csub = DC // nsub          # chunks per x sub-load

    nc = bacc.Bacc("TRN2", target_bir_lowering=False)
    # host-tiled layouts (see prepare_in_maps): row p of group-block h holds
    # all of partition p's data for that group, contiguous.
    xhd = nc.dram_tensor("xt_hi", [nh * P, DC * th], BF16, kind="ExternalInput")
    xld = nc.dram_tensor("xt_lo", [nh * P, DC * th], BF16, kind="ExternalInput")
    whd = nc.dram_tensor("wt_hi", [P, DC * E], BF16, kind="ExternalInput")
    wld = nc.dram_tensor("wt_lo", [P, DC * E], BF16, kind="ExternalInput")
    b = nc.dram_tensor("bias", [1, E], F32, kind="ExternalInput")
    oi = nc.dram_tensor("out_idx", [ts, K], mybir.dt.int32, kind="ExternalOutput")
    ow = nc.dram_tensor("out_w", [ts, K], F32, kind="ExternalOutput")

    with TileContext(nc) as tc:
        with (
            tc.tile_pool(name="const", bufs=1) as cpool,
            tc.tile_pool(name="xin", bufs=nh) as xpool,
            tc.tile_pool(name="epi", bufs=2) as epool,
            tc.tile_pool(name="outb", bufs=2) as opool,
            tc.tile_pool(name="plogit", bufs=2, space="PSUM") as plogit,
        ):
            bias_row = cpool.tile([1, E], F32)
            nc.gpsimd.dma_start(out=bias_row, in_=b[:, :])
            bias_bc = cpool.tile([P, E], F32)
            nc.gpsimd.partition_broadcast(bias_bc, bias_row)

            # w loads + x lo-loads ride the Activation HWDGE queue, x hi-loads
            # the SP queue: two hardware queues in parallel.
            wth = cpool.tile([P, DC, E], BF16)
            nc.scalar.dma_start(out=wth, in_=whd[:, :])
            wtl = cpool.tile([P, DC, E], BF16)
            nc.scalar.dma_start(out=wtl, in_=wld[:, :])

            # queue ALL x loads up front so no compute op ever blocks a DMA
            # trigger behind it on either HWDGE sequencer (SP: hi, ACT: lo)
            xhts, xlts = [], []
            for h in range(nh):
                xht = xpool.tile([P, DC, th], BF16, tag="xh", name=f"xh{h}")
                xlt = xpool.tile([P, DC, th], BF16, tag="xl", name=f"xl{h}")
                for s in range(nsub):
                    sl = slice(s * csub * th, (s + 1) * csub * th)
                    nc.sync.dma_start(
                        out=xht[:].rearrange("p c t -> p (c t)")[:, sl],
                        in_=xhd[h * P:(h + 1) * P, sl],
                    )
                    nc.scalar.dma_start(
                        out=xlt[:].rearrange("p c t -> p (c t)")[:, sl],
                        in_=xld[h * P:(h + 1) * P, sl],
                    )
                xhts.append(xht)
                xlts.append(xlt)

            for h in range(nh):
                xht, xlt = xhts[h], xlts[h]
                lgs = [
                    plogit.tile([P, E], F32, tag=f"lg{i}", name=f"lg_h{h}_{i}")
                    for i in range(ntg)
                ]
                for c in range(DC):
                    for i, lg in enumerate(lgs):
                        tsl = slice(i * P, (i + 1) * P)
                        nc.tensor.matmul(
                            lg, xht[:, c, tsl], wth[:, c, :],
                            start=(c == 0), stop=False,
                        )
                        nc.tensor.matmul(
                            lg, xht[:, c, tsl], wtl[:, c, :],
                            start=False, stop=False,
                        )
                        nc.tensor.matmul(
                            lg, xlt[:, c, tsl], wth[:, c, :],
                            start=False, stop=(c == DC - 1),
                        )

                oidx = opool.tile([P, ntg, K], mybir.dt.uint32, name=f"oidx{h}")
                owgt = opool.tile([P, ntg, K], F32, name=f"owgt{h}")
                for i, lg in enumerate(lgs):
                    # sigmoid = 1 / (1 + e^-l)
                    ex = epool.tile([P, E], F32, tag="ex")
                    nc.scalar.activation(
                        out=ex, in_=lg,
                        func=mybir.ActivationFunctionType.Exp, scale=-1.0,
                    )
                    den = epool.tile([P, E], F32, tag="den")
                    nc.vector.tensor_scalar_add(den, ex, 1.0)
                    sc = epool.tile([P, E], F32, tag="sc")
                    nc.vector.reciprocal(out=sc, in_=den)
                    bi = epool.tile([P, E], F32, tag="bi")
                    nc.vector.tensor_tensor(
                        out=bi, in0=sc, in1=bias_bc, op=mybir.AluOpType.add
                    )
                    msel = epool.tile([P, K], F32, tag="msel")
                    nc.vector.max(out=msel, in_=bi)
                    nc.vector.max_index(
                        out=oidx[:, i, :], in_max=msel, in_values=bi
                    )
                    ssel = epool.tile([P, K], F32, tag="ssel")
                    for k in range(K):
                        scr = epool.tile([P, E], F32, tag="scr")
                        nc.vector.scalar_tensor_tensor(
                            out=scr,
                            in0=bi,
                            scalar=msel[:, k:k + 1],
                            in1=sc,
                            op0=mybir.AluOpType.is_equal,
                            op1=mybir.AluOpType.mult,
                            accum_out=ssel[:, k:k + 1],
                        )
                    ssum = epool.tile([P, 1], F32, tag="ssum")
                    nc.vector.tensor_reduce(
                        out=ssum, in_=ssel,
                        axis=mybir.AxisListType.X, op=mybir.AluOpType.add,
                    )
                    rsum = epool.tile([P, 1], F32, tag="rsum")
                    nc.vector.reciprocal(out=rsum, in_=ssum)
                    nc.vector.tensor_scalar_mul(owgt[:, i, :], ssel, rsum[:])

                # token at output partition q of tile i is h*th + q*ntg + i,
                # so rows of oi/ow group-slices are per-partition contiguous
                nc.sync.dma_start(
                    out=oi[h * th:(h + 1) * th, :].rearrange(
                        "(q i) k -> q i k", i=ntg
                    ),
                    in_=oidx[:].bitcast(mybir.dt.int32),
                )
                nc.sync.dma_start(
                    out=ow[h * th:(h + 1) * th, :].rearrange(
                        "(q i) k -> q i k", i=ntg
                    ),
                    in_=owgt,
                )

    nc.compile()
    return nc


_NC_CACHE = {}


def _get_nc(ts):
    if ts not in _NC_CACHE:
        _NC_CACHE[ts] = build_nc(ts)
    return _NC_CACHE[ts]


def _tile_xt(xs, nh, th, ntg):
    """[ts, D] fp32 -> [nh*P, DC*th] fp32 in the device layout.

    Group h, partition row p holds x[h*th + q*ntg + i, c*P + p] at flat
    column c*th + i*P + q.
    """
    ts = xs.shape[0]
    # [h, q, i, c, p] <- token h*th + q*ntg + i, feature c*P + p
    a = xs.reshape(nh, P, ntg, DC, P)          # [h, q, i, c, p]
    a = a.transpose(0, 4, 3, 2, 1)             # [h, p, c, i, q]
    return np.ascontiguousarray(a).reshape(nh * P, DC * th)


def prepare_in_maps(x, gate_weight, bias):
    x = np.asarray(x, dtype=np.float32)
    gw = np.asarray(gate_weight, dtype=np.float32)
    bb = np.ascontiguousarray(np.asarray(bias, dtype=np.float32)).reshape(1, E)

    ts = T // NCORES
    nh = NH if ts >= NH * P else max(1, ts // P)
    th = ts // nh
    ntg = th // P

    # W^T in device layout [P, DC*E]: [p, c*E + e] = W[e, c*P + p]
    wt = np.ascontiguousarray(gw.T.reshape(DC, P, E).transpose(1, 0, 2)).reshape(
        P, DC * E
    )
    wh = wt.astype(BF16_NP)
    wl = (wt - wh.astype(np.float32)).astype(BF16_NP)

    in_maps = []
    for cid in range(NCORES):
        xs = x[cid * ts:(cid + 1) * ts]
        xt = _tile_xt(xs, nh, th, ntg)
        xh = xt.astype(BF16_NP)
        xl = (xt - xh.astype(np.float32)).astype(BF16_NP)
        in_maps.append({
            "xt_hi": xh,
            "xt_lo": xl,
            "wt_hi": wh,
            "wt_lo": wl,
            "bias": bb,
        })
    return in_maps


def kernel(x, gate_weight, bias):
    ts = T // NCORES
    nc = _get_nc(ts)
    in_maps = prepare_in_maps(x, gate_weight, bias)
    res = run_bass_kernel_spmd(nc, in_maps, core_ids=list(range(NCORES)))
    idx = np.concatenate([r["out_idx"] for r in res.results], axis=0)
    wts = np.concatenate([r["out_w"] for r in res.results], axis=0)
    return idx, wts
